# revision 20
# baseline (speedup 1.0000x reference)
"""BitNet-style quantized linear layer on 8 Trainium2 NeuronCores.

Reference semantics (fp32):
    x_scale = clip(max|x| over last dim, 1e-5)          # per row of x
    x_quant = clip(round(x / x_scale * 127), -128, 127)
    w_mean  = mean(weight); w_c = weight - w_mean
    w_scale = clip(mean|w_c|, 1e-5)
    w_quant = clip(round(w_c / w_scale), -1, 1)         # ternary
    y = (x_quant @ w_quant.T) * (w_scale * x_scale / 127)

Sharding / data movement (optimized for end-to-end wall time):
  * x rows are data-parallel: global [16384, 2048] sharded on axis 0 across
    8 cores (2048 rows/core) -- shipped host->device with no host-side copy.
  * weight is sharded by out_features rows (256/core, 2 MiB each) -- the
    full 16 MiB is shipped exactly once instead of 8x-replicated.
  * w_mean / w_scale are computed on-device with two tiny AllReduces; each
    core ternary-quantizes its weight shard, and the bf16 quantized weight
    (8 MiB) is AllGathered so every core has the full w_quant.
  * output is one global [16384, 2048] f32 sharded on axis 0; a single
    device->host fetch, reshaped (no host concat).
  * dispatch goes through a direct shard_map wrapper around the bass_exec
    primitive (same machinery run_bass_kernel_spmd uses under axon), with
    no donated zero output buffers -- the kernel writes every element.

Exactness notes:
  * matmul runs in bf16: x_quant in [-127,127] and ternary w_quant are both
    exactly representable in bf16, and PSUM accumulates fp32 exactly
    (|partial sums| < 2^19), so the GEMM is bit-exact.
  * ternary quant is computed as (t > tau) + (t >= -tau) - 1 with
    tau = 0.5*w_scale (exact fp32), matching round-half-even of t/s clipped
    to [-1,1] bit-for-bit.
  * w stats are computed in fp32 on-device (row reduce + partition reduce +
    8-way AllReduce); the ~1e-7 relative difference vs the host/f64 path
    can flip ternary values only for weights within ~1e-7*tau of the
    threshold (expected < 1 element of 4.2M; each flip perturbs y by
    < 1% of max|y|, far below the 2e-2 tolerance).
  * x quant uses 127/x_scale via DVE reciprocal + the (v + 1.5*2^23) -
    1.5*2^23 round-half-even trick; divergence vs the reference's divide is
    ~1e-7 relative and only perturbs |q_x| by +-1 on ~1e-5 of elements.
"""

import numpy as np

R_TOTAL = 16384  # B * S
D = 2048         # D_IN == D_OUT
N_CORES = 8
R_CORE = R_TOTAL // N_CORES   # 2048 rows per core
O_SHARD = D // N_CORES        # 256 weight rows (out features) per core
NS = O_SHARD // 128           # 2 shard strips
NK = D // 128                 # 16 contraction strips
NR = R_CORE // 128            # 16 row tiles per core
NO = D // 512                 # 4 output banks of 512
MAGIC = float(1.5 * 2 ** 23)  # round-half-even offset (ulp=1 both sides)
NELEM = float(D * D)          # weight element count

_FN_CACHE = {}
_NC = None           # compiled Bass program (test harness introspection)
LAST_RESULTS = None  # test harness peeks at this for profiling info


def _emit(nc, tc, ctx, xs_ap, wt_ap, ys_ap, gql, gqa):
    """Emit one full forward pass (per-core program body)."""
    import concourse.mybir as mybir
    from concourse import masks
    from concourse import bass_isa

    f32 = mybir.dt.float32
    bf16 = mybir.dt.bfloat16
    Alu = mybir.AluOpType
    Act = mybir.ActivationFunctionType

    # DRAM bounce tiles for the two stat AllReduces (+ a warmup dummy)
    arin0 = nc.dram_tensor("arin0", [1, 16], f32, kind="Internal")
    arout0 = nc.dram_tensor("arout0", [1, 16], f32, kind="Internal",
                            addr_space="Shared")
    arin1 = nc.dram_tensor("arin1", [1, 16], f32, kind="Internal")
    arout1 = nc.dram_tensor("arout1", [1, 16], f32, kind="Internal",
                            addr_space="Shared")
    arin2 = nc.dram_tensor("arin2", [1, 16], f32, kind="Internal")
    arout2 = nc.dram_tensor("arout2", [1, 16], f32, kind="Internal",
                            addr_space="Shared")

    # warm up the collectives path right away: the first collective pays
    # ~25us of ncfw startup; burn it on a dummy while local stats compute
    nc.gpsimd.collective_compute(
        "AllReduce", mybir.AluOpType.add,
        replica_groups=[list(range(N_CORES))],
        ins=[arin0.ap().opt()], outs=[arout0.ap().opt()])

    cpool = ctx.enter_context(tc.tile_pool(name="cpool", bufs=1))
    ident = cpool.tile([128, 128], bf16)
    masks.make_identity(nc, ident[:])

    # ---- weight shard load + global stats + ternary quant + AllGather ----
    wpool = ctx.enter_context(tc.tile_pool(name="wpool", bufs=1))
    stp = ctx.enter_context(tc.tile_pool(name="stats", bufs=1))

    wsb = []
    for i in range(NS):
        w = wpool.tile([128, D], f32, name=f"wsb{i}", tag=f"wsb{i}")
        nc.scalar.dma_start(w[:], wt_ap[i * 128:(i + 1) * 128, :])
        wsb.append(w)

    # pass 1: sum(w) over the local shard
    srow = stp.tile([128, 2], f32, name="srow")
    for i in range(NS):
        nc.vector.tensor_reduce(srow[:, i:i + 1], wsb[i][:],
                                axis=mybir.AxisListType.X, op=Alu.add)
    s01 = stp.tile([128, 1], f32, name="s01")
    nc.vector.tensor_reduce(s01[:], srow[:], axis=mybir.AxisListType.X,
                            op=Alu.add)
    sred = stp.tile([128, 1], f32, name="sred")
    nc.gpsimd.partition_all_reduce(sred[:], s01[:], channels=128,
                                   reduce_op=bass_isa.ReduceOp.add)
    nc.scalar.dma_start(arin1.ap()[0:1, 0:1], sred[0:1, 0:1])
    nc.gpsimd.collective_compute(
        "AllReduce", Alu.add, replica_groups=[list(range(N_CORES))],
        ins=[arin1.ap().opt()], outs=[arout1.ap().opt()])
    msum = stp.tile([1, 16], f32, name="msum")
    nc.scalar.dma_start(msum[0:1, :], arout1.ap()[0:1, :])
    # negm = -mean = msum * (-1/NELEM)
    negm1 = stp.tile([1, 1], f32, name="negm1")
    nc.vector.tensor_scalar(negm1[0:1, :], msum[0:1, 0:1], -1.0 / NELEM,
                            None, op0=Alu.mult)
    negmb = stp.tile([128, 1], f32, name="negmb")
    nc.gpsimd.partition_broadcast(negmb[:], negm1[0:1, :], channels=128)

    # pass 2: center, then sum|w - mean|
    wcen = []
    arow = stp.tile([128, 2], f32, name="arow")
    for i in range(NS):
        wc = wpool.tile([128, D], f32, name=f"wcen{i}", tag=f"wcen{i}")
        nc.scalar.activation(wc[:], wsb[i][:], Act.Identity,
                             bias=negmb[:, 0:1], scale=1.0)
        nc.vector.tensor_reduce(arow[:, i:i + 1], wc[:],
                                axis=mybir.AxisListType.X, op=Alu.add,
                                apply_absolute_value=True)
        wcen.append(wc)
    a01 = stp.tile([128, 1], f32, name="a01")
    nc.vector.tensor_reduce(a01[:], arow[:], axis=mybir.AxisListType.X,
                            op=Alu.add)
    ared = stp.tile([128, 1], f32, name="ared")
    nc.gpsimd.partition_all_reduce(ared[:], a01[:], channels=128,
                                   reduce_op=bass_isa.ReduceOp.add)
    nc.scalar.dma_start(arin2.ap()[0:1, 0:1], ared[0:1, 0:1])
    nc.gpsimd.collective_compute(
        "AllReduce", Alu.add, replica_groups=[list(range(N_CORES))],
        ins=[arin2.ap().opt()], outs=[arout2.ap().opt()])
    asum = stp.tile([1, 16], f32, name="asum")
    nc.scalar.dma_start(asum[0:1, :], arout2.ap()[0:1, :])
    # w_scale = clip(asum/NELEM, 1e-5); vec = [tau, -tau, ws/127]
    wsc = stp.tile([1, 1], f32, name="wsc")
    nc.vector.tensor_scalar(wsc[0:1, :], asum[0:1, 0:1], 1.0 / NELEM, 1e-5,
                            op0=Alu.mult, op1=Alu.max)
    vec3 = stp.tile([1, 3], f32, name="vec3")
    nc.vector.tensor_scalar(vec3[0:1, 0:1], wsc[0:1, :], 0.5, None,
                            op0=Alu.mult)
    nc.vector.tensor_scalar(vec3[0:1, 1:2], wsc[0:1, :], -0.5, None,
                            op0=Alu.mult)
    nc.vector.tensor_scalar(vec3[0:1, 2:3], wsc[0:1, :],
                            float(1.0 / np.float32(127.0)), None,
                            op0=Alu.mult)
    c_sb = stp.tile([128, 3], f32, name="c_sb")
    nc.gpsimd.partition_broadcast(c_sb[:], vec3[0:1, :], channels=128)
    tau = c_sb[:, 0:1]
    neg_tau = c_sb[:, 1:2]
    ws127 = c_sb[:, 2:3]

    # ternary quantize the local shard (bit-exact; see module docstring)
    for i in range(NS):
        wa2 = wsb[i]  # shard raw strip is dead now; reuse as scratch
        nc.vector.tensor_scalar(wa2[:], wcen[i][:], neg_tau, -1.0,
                                op0=Alu.is_ge, op1=Alu.add)
        wqs = wpool.tile([128, D], bf16, name=f"wqs{i}", tag=f"wqs{i}")
        nc.vector.scalar_tensor_tensor(wqs[:], wcen[i][:], tau, wa2[:],
                                       op0=Alu.is_gt, op1=Alu.add)
        nc.scalar.dma_start(gql[0].ap()[i * 128:(i + 1) * 128, :],
                            wqs[:, 0:D // 2])
        nc.scalar.dma_start(gql[1].ap()[i * 128:(i + 1) * 128, :],
                            wqs[:, D // 2:D])

    # gather the full ternary weight (bf16, [out_features, in_features]),
    # split along K so the first half's strip loads + matmuls can start
    # while the second half is still on the wire
    wq_pool = ctx.enter_context(tc.tile_pool(name="wqp", bufs=1))
    wq = [None] * NK
    for h in range(2):
        nc.gpsimd.collective_compute(
            "AllGather", Alu.bypass, replica_groups=[list(range(N_CORES))],
            ins=[gql[h].ap().opt()], outs=[gqa[h].ap().opt()])
        for kk in range(NK // 2):
            k = h * (NK // 2) + kk
            wqk = wq_pool.tile([128, D], bf16, name=f"wq{k}", tag=f"wq{k}")
            nc.scalar.dma_start_transpose(
                wqk[:], gqa[h].ap()[:, kk * 128:(kk + 1) * 128])
            wq[k] = wqk

    # ---- x pipeline pools ----
    x_pool = ctx.enter_context(tc.tile_pool(name="xp", bufs=3))
    st_pool = ctx.enter_context(tc.tile_pool(name="stp", bufs=6))
    xq_pool = ctx.enter_context(tc.tile_pool(name="xqp", bufs=3))
    xqT_pool = ctx.enter_context(tc.tile_pool(name="xqTp", bufs=4))
    tp_psum = ctx.enter_context(
        tc.tile_pool(name="tpps", bufs=2, space="PSUM"))
    y_psum = ctx.enter_context(
        tc.tile_pool(name="yps", bufs=6, space="PSUM"))
    y_pool = ctx.enter_context(tc.tile_pool(name="yop", bufs=2))

    def emit_x_prep(r):
        """Load + quantize + transpose one 128-row tile; returns (xqT, comb)."""
        xr = x_pool.tile([128, D], f32, name="xr")
        nc.sync.dma_start(xr[:], xs_ap[r * 128:(r + 1) * 128, :])

        mx = st_pool.tile([128, 1], f32, name="mx")
        nc.vector.tensor_reduce(mx[:], xr[:], axis=mybir.AxisListType.X,
                                op=Alu.max, apply_absolute_value=True)
        mxc = st_pool.tile([128, 1], f32, name="mxc")
        nc.vector.tensor_scalar(mxc[:], mx[:], 1e-5, None, op0=Alu.max)
        rec = st_pool.tile([128, 1], f32, name="rec")
        nc.vector.reciprocal(rec[:], mxc[:])
        comb = st_pool.tile([128, 1], f32, name="comb")
        nc.vector.tensor_scalar(comb[:], mxc[:], ws127, None, op0=Alu.mult)

        # xq = round_half_even(x * (1/s) * 127) in bf16
        xsc = x_pool.tile([128, D], f32, name="xsc")
        nc.vector.tensor_scalar(xsc[:], xr[:], rec, 127.0,
                                op0=Alu.mult, op1=Alu.mult)
        xq = xq_pool.tile([128, D], bf16, name="xq")
        nc.vector.tensor_scalar(xq[:], xsc[:], MAGIC, MAGIC,
                                op0=Alu.add, op1=Alu.subtract)

        # transpose xq via PE into xqT (bf16), strip by strip
        xqT = xqT_pool.tile([128, D], bf16, name="xqT")
        for h in range(2):
            pst = tp_psum.tile([128, 1024], bf16, name="pst")
            for kk in range(8):
                k = h * 8 + kk
                nc.tensor.transpose(pst[:, kk * 128:(kk + 1) * 128],
                                    xq[:, k * 128:(k + 1) * 128],
                                    ident[:])
            nc.scalar.copy(xqT[:, h * 1024:(h + 1) * 1024], pst[:])
        return xqT, comb

    def emit_mms(r, xqT, yps, ks):
        """k-outer / o-inner: one LDWEIGHTS per k feeds 4 o-bank matmuls."""
        for k in ks:
            for o in range(NO):
                nc.tensor.matmul(yps[o][:],
                                 xqT[:, k * 128:(k + 1) * 128],
                                 wq[k][:, o * 512:(o + 1) * 512],
                                 start=(k == 0), stop=(k == NK - 1))

    def emit_finish(r, yps, comb):
        ysb = y_pool.tile([128, D], f32, name="ysb")
        for o in range(NO):
            # dequant rescale: y * (w_scale * x_scale / 127)
            nc.scalar.mul(ysb[:, o * 512:(o + 1) * 512], yps[o][:], comb)
        nc.sync.dma_start(ys_ap[r * 128:(r + 1) * 128, :], ysb[:])

    def alloc_psum(r):
        # yp0/yp1 double-buffered so the next r-tile's first matmuls can
        # start while this one's rescales drain: 2+2+1+1 (+2 pst) = 8 banks
        return [y_psum.tile([128, 512], f32, name=f"yp{o}", tag=f"yp{o}",
                            bufs=(2 if o < 2 else 1))
                for o in range(NO)]

    # ---- prologue: r0/r1 x-prep first (overlaps the weight/collective ----
    # ---- path), then r0's and r1's matmuls as wq strips land          ----
    xqT0, comb0 = emit_x_prep(0)
    xqT1, comb1 = emit_x_prep(1)
    yps0 = alloc_psum(0)
    yps1 = alloc_psum(1)
    for k in range(NK):
        emit_mms(0, xqT0, yps0, [k])
        for o in range(2):  # r1 uses the double-buffered yp0/yp1 slots
            nc.tensor.matmul(yps1[o][:],
                             xqT1[:, k * 128:(k + 1) * 128],
                             wq[k][:, o * 512:(o + 1) * 512],
                             start=(k == 0), stop=(k == NK - 1))
    xqT2, comb2 = emit_x_prep(2)
    emit_finish(0, yps0, comb0)
    # r1's remaining o-banks (2nd k-pass), then finish r1
    for k in range(NK):
        for o in range(2, NO):
            nc.tensor.matmul(yps1[o][:],
                             xqT1[:, k * 128:(k + 1) * 128],
                             wq[k][:, o * 512:(o + 1) * 512],
                             start=(k == 0), stop=(k == NK - 1))
    xqT3, comb3 = emit_x_prep(3)
    emit_finish(1, yps1, comb1)

    # ---- steady state: two-stage software pipeline ----
    pending = [(2, xqT2, comb2), (3, xqT3, comb3)]
    for r in range(4, NR + 2):
        pr, pxqT, pcomb = pending.pop(0)
        yps = alloc_psum(pr)
        emit_mms(pr, pxqT, yps, range(NK))
        if r < NR:
            pending.append((r, *emit_x_prep(r)))
        emit_finish(pr, yps, pcomb)


def _build_program():
    import concourse.bacc as bacc
    import concourse.mybir as mybir
    import concourse.tile as tile
    from contextlib import ExitStack

    f32 = mybir.dt.float32
    bf16 = mybir.dt.bfloat16
    nc = bacc.Bacc("TRN2", target_bir_lowering=False, debug=False,
                   num_devices=N_CORES)

    xs = nc.dram_tensor("xs", [R_CORE, D], f32, kind="ExternalInput")
    wt = nc.dram_tensor("wt", [O_SHARD, D], f32, kind="ExternalInput")
    ys = nc.dram_tensor("ys", [R_CORE, D], f32, kind="ExternalOutput")
    gqlA = nc.dram_tensor("gqlA", [O_SHARD, D // 2], bf16, kind="Internal")
    gqlB = nc.dram_tensor("gqlB", [O_SHARD, D // 2], bf16, kind="Internal")
    gqaA = nc.dram_tensor("gqaA", [D, D // 2], bf16, kind="Internal",
                          addr_space="Shared")
    gqaB = nc.dram_tensor("gqaB", [D, D // 2], bf16, kind="Internal",
                          addr_space="Shared")

    with tile.TileContext(nc) as tc, ExitStack() as ctx:
        _emit(nc, tc, ctx, xs.ap(), wt.ap(), ys.ap(),
              (gqlA, gqlB), (gqaA, gqaB))

    nc.compile()
    return nc


def _build_dispatch():
    """jit(shard_map) wrapper around bass_exec: global arrays sharded on
    axis 0, no host-side concat, no donated zero output buffers."""
    import jax
    from jax.sharding import Mesh, PartitionSpec
    from jax.experimental.shard_map import shard_map
    import concourse.mybir as mybir
    from concourse import bass2jax

    global _NC
    nc = _build_program()
    _NC = nc

    partition_name = (nc.partition_id_tensor.name
                      if nc.partition_id_tensor else None)
    in_names, out_names, out_avals = [], [], []
    for alloc in nc.m.functions[0].allocations:
        if not isinstance(alloc, mybir.MemoryLocationSet):
            continue
        name = alloc.memorylocations[0].name
        if alloc.kind == "ExternalInput":
            if name != partition_name:
                in_names.append(name)
        elif alloc.kind == "ExternalOutput":
            out_names.append(name)
            out_avals.append(jax.core.ShapedArray(
                tuple(alloc.tensor_shape), mybir.dt.np(alloc.dtype)))
    if partition_name is not None:
        in_names.append(partition_name)
    assert in_names[:2] == ["xs", "wt"] and out_names == ["ys"], (
        in_names, out_names)

    bass2jax.install_neuronx_cc_hook()

    def _body(*args):
        operands = list(args)
        if partition_name is not None:
            operands.append(bass2jax.partition_id_tensor())
        outs = bass2jax._bass_exec_p.bind(
            *operands,
            out_avals=tuple(out_avals),
            in_names=tuple(in_names),
            out_names=tuple(out_names),
            lowering_input_output_aliases=(),
            sim_require_finite=True,
            sim_require_nnan=True,
            nc=nc,
        )
        return tuple(outs)

    devices = jax.devices()[:N_CORES]
    mesh = Mesh(np.asarray(devices), ("core",))
    spec = PartitionSpec("core")

    def _make_jit():
        return jax.jit(shard_map(_body, mesh=mesh, in_specs=(spec, spec),
                                 out_specs=(spec,), check_rep=False))

    try:
        # AOT-compile with bass_effect suppressed: the jit call takes the
        # C++ fast dispatch path instead of the python effects path.
        from jax.sharding import NamedSharding
        sh = NamedSharding(mesh, spec)
        x_sds = jax.ShapeDtypeStruct((R_TOTAL, D), np.float32, sharding=sh)
        w_sds = jax.ShapeDtypeStruct((D, D), np.float32, sharding=sh)
        return bass2jax.fast_dispatch_compile(
            lambda: _make_jit().lower(x_sds, w_sds).compile())
    except Exception:
        return _make_jit()


def _get_fn():
    key = (R_CORE, D)
    if key not in _FN_CACHE:
        _FN_CACHE[key] = _build_dispatch()
    return _FN_CACHE[key]


def kernel(x: np.ndarray, weight: np.ndarray, _trace: bool = False,
           **_unused) -> np.ndarray:
    global LAST_RESULTS
    LAST_RESULTS = None

    x = np.asarray(x)
    weight = np.asarray(weight)
    orig_shape = x.shape
    x2d = np.ascontiguousarray(
        x.reshape(R_TOTAL, D).astype(np.float32, copy=False))
    w = np.ascontiguousarray(weight.astype(np.float32, copy=False))

    fn = _get_fn()
    out = fn(x2d, w)[0]
    y2d = np.asarray(out)
    return y2d.reshape(orig_shape).astype(np.float32, copy=False)


# revision 21
# speedup vs baseline: 1.1642x; 1.1642x over previous
"""BitNet-style quantized linear layer on 8 Trainium2 NeuronCores.

Reference semantics (fp32):
    x_scale = clip(max|x| over last dim, 1e-5)          # per row of x
    x_quant = clip(round(x / x_scale * 127), -128, 127)
    w_mean  = mean(weight); w_c = weight - w_mean
    w_scale = clip(mean|w_c|, 1e-5)
    w_quant = clip(round(w_c / w_scale), -1, 1)         # ternary
    y = (x_quant @ w_quant.T) * (w_scale * x_scale / 127)

Sharding / data movement (optimized for end-to-end wall time):
  * x rows are data-parallel: global [16384, 2048] sharded on axis 0 across
    8 cores (2048 rows/core) -- shipped host->device with no host-side copy.
  * weight is sharded by out_features rows (256/core, 2 MiB each) -- the
    full 16 MiB is shipped exactly once instead of 8x-replicated.
  * w_mean / w_scale are computed on-device with two tiny AllReduces; each
    core ternary-quantizes its weight shard, and the bf16 quantized weight
    (8 MiB) is AllGathered so every core has the full w_quant.
  * output is one global [16384, 2048] f32 sharded on axis 0; a single
    device->host fetch, reshaped (no host concat).
  * dispatch goes through a direct shard_map wrapper around the bass_exec
    primitive (same machinery run_bass_kernel_spmd uses under axon), with
    no donated zero output buffers -- the kernel writes every element.

Exactness notes:
  * matmul runs in bf16: x_quant in [-127,127] and ternary w_quant are both
    exactly representable in bf16, and PSUM accumulates fp32 exactly
    (|partial sums| < 2^19), so the GEMM is bit-exact.
  * ternary quant is computed as (t > tau) + (t >= -tau) - 1 with
    tau = 0.5*w_scale (exact fp32), matching round-half-even of t/s clipped
    to [-1,1] bit-for-bit.
  * w stats are computed in fp32 on-device (row reduce + partition reduce +
    8-way AllReduce); the ~1e-7 relative difference vs the host/f64 path
    can flip ternary values only for weights within ~1e-7*tau of the
    threshold (expected < 1 element of 4.2M; each flip perturbs y by
    < 1% of max|y|, far below the 2e-2 tolerance).
  * x quant uses 127/x_scale via DVE reciprocal + the (v + 1.5*2^23) -
    1.5*2^23 round-half-even trick; divergence vs the reference's divide is
    ~1e-7 relative and only perturbs |q_x| by +-1 on ~1e-5 of elements.
"""

import numpy as np

R_TOTAL = 16384  # B * S
D = 2048         # D_IN == D_OUT
N_CORES = 8
R_CORE = R_TOTAL // N_CORES   # 2048 rows per core
O_SHARD = D // N_CORES        # 256 weight rows (out features) per core
NS = O_SHARD // 128           # 2 shard strips
NK = D // 128                 # 16 contraction strips
NR = R_CORE // 128            # 16 row tiles per core
NO = D // 512                 # 4 output banks of 512
MAGIC = float(1.5 * 2 ** 23)  # round-half-even offset (ulp=1 both sides)
NELEM = float(D * D)          # weight element count

_FN_CACHE = {}
_NC = None           # compiled Bass program (test harness introspection)
LAST_RESULTS = None  # test harness peeks at this for profiling info


def _emit(nc, tc, ctx, xs_ap, wt_ap, ys_ap, gql, gqa):
    """Emit one full forward pass (per-core program body)."""
    import concourse.mybir as mybir
    from concourse import masks
    from concourse import bass_isa

    f32 = mybir.dt.float32
    bf16 = mybir.dt.bfloat16
    Alu = mybir.AluOpType
    Act = mybir.ActivationFunctionType

    # DRAM bounce tiles for the two stat AllReduces (+ a warmup dummy)
    arin0 = nc.dram_tensor("arin0", [1, 16], f32, kind="Internal")
    arout0 = nc.dram_tensor("arout0", [1, 16], f32, kind="Internal",
                            addr_space="Shared")
    arin1 = nc.dram_tensor("arin1", [1, 16], f32, kind="Internal")
    arout1 = nc.dram_tensor("arout1", [1, 16], f32, kind="Internal",
                            addr_space="Shared")
    arin2 = nc.dram_tensor("arin2", [1, 16], f32, kind="Internal")
    arout2 = nc.dram_tensor("arout2", [1, 16], f32, kind="Internal",
                            addr_space="Shared")

    # warm up the collectives path right away: the first collective pays
    # ~25us of ncfw startup; burn it on a dummy while local stats compute
    nc.gpsimd.collective_compute(
        "AllReduce", mybir.AluOpType.add,
        replica_groups=[list(range(N_CORES))],
        ins=[arin0.ap().opt()], outs=[arout0.ap().opt()])

    cpool = ctx.enter_context(tc.tile_pool(name="cpool", bufs=1))
    ident = cpool.tile([128, 128], bf16)
    masks.make_identity(nc, ident[:])

    # ---- weight shard load + global stats + ternary quant + AllGather ----
    wpool = ctx.enter_context(tc.tile_pool(name="wpool", bufs=1))
    stp = ctx.enter_context(tc.tile_pool(name="stats", bufs=1))

    wsb = []
    for i in range(NS):
        w = wpool.tile([128, D], f32, name=f"wsb{i}", tag=f"wsb{i}")
        nc.scalar.dma_start(w[:], wt_ap[i * 128:(i + 1) * 128, :])
        wsb.append(w)

    # pass 1: sum(w) over the local shard
    srow = stp.tile([128, 2], f32, name="srow")
    for i in range(NS):
        nc.vector.tensor_reduce(srow[:, i:i + 1], wsb[i][:],
                                axis=mybir.AxisListType.X, op=Alu.add)
    s01 = stp.tile([128, 1], f32, name="s01")
    nc.vector.tensor_reduce(s01[:], srow[:], axis=mybir.AxisListType.X,
                            op=Alu.add)
    sred = stp.tile([128, 1], f32, name="sred")
    nc.gpsimd.partition_all_reduce(sred[:], s01[:], channels=128,
                                   reduce_op=bass_isa.ReduceOp.add)
    nc.scalar.dma_start(arin1.ap()[0:1, 0:1], sred[0:1, 0:1])
    nc.gpsimd.collective_compute(
        "AllReduce", Alu.add, replica_groups=[list(range(N_CORES))],
        ins=[arin1.ap().opt()], outs=[arout1.ap().opt()])
    msum = stp.tile([1, 16], f32, name="msum")
    nc.scalar.dma_start(msum[0:1, :], arout1.ap()[0:1, :])
    # negm = -mean = msum * (-1/NELEM)
    negm1 = stp.tile([1, 1], f32, name="negm1")
    nc.vector.tensor_scalar(negm1[0:1, :], msum[0:1, 0:1], -1.0 / NELEM,
                            None, op0=Alu.mult)
    negmb = stp.tile([128, 1], f32, name="negmb")
    nc.gpsimd.partition_broadcast(negmb[:], negm1[0:1, :], channels=128)

    # pass 2: center, then sum|w - mean|
    wcen = []
    arow = stp.tile([128, 2], f32, name="arow")
    for i in range(NS):
        wc = wpool.tile([128, D], f32, name=f"wcen{i}", tag=f"wcen{i}")
        nc.scalar.activation(wc[:], wsb[i][:], Act.Identity,
                             bias=negmb[:, 0:1], scale=1.0)
        nc.vector.tensor_reduce(arow[:, i:i + 1], wc[:],
                                axis=mybir.AxisListType.X, op=Alu.add,
                                apply_absolute_value=True)
        wcen.append(wc)
    a01 = stp.tile([128, 1], f32, name="a01")
    nc.vector.tensor_reduce(a01[:], arow[:], axis=mybir.AxisListType.X,
                            op=Alu.add)
    ared = stp.tile([128, 1], f32, name="ared")
    nc.gpsimd.partition_all_reduce(ared[:], a01[:], channels=128,
                                   reduce_op=bass_isa.ReduceOp.add)
    nc.scalar.dma_start(arin2.ap()[0:1, 0:1], ared[0:1, 0:1])
    nc.gpsimd.collective_compute(
        "AllReduce", Alu.add, replica_groups=[list(range(N_CORES))],
        ins=[arin2.ap().opt()], outs=[arout2.ap().opt()])
    asum = stp.tile([1, 16], f32, name="asum")
    nc.scalar.dma_start(asum[0:1, :], arout2.ap()[0:1, :])
    # w_scale = clip(asum/NELEM, 1e-5); vec = [tau, -tau, ws/127]
    wsc = stp.tile([1, 1], f32, name="wsc")
    nc.vector.tensor_scalar(wsc[0:1, :], asum[0:1, 0:1], 1.0 / NELEM, 1e-5,
                            op0=Alu.mult, op1=Alu.max)
    vec3 = stp.tile([1, 3], f32, name="vec3")
    nc.vector.tensor_scalar(vec3[0:1, 0:1], wsc[0:1, :], 0.5, None,
                            op0=Alu.mult)
    nc.vector.tensor_scalar(vec3[0:1, 1:2], wsc[0:1, :], -0.5, None,
                            op0=Alu.mult)
    nc.vector.tensor_scalar(vec3[0:1, 2:3], wsc[0:1, :],
                            float(1.0 / np.float32(127.0)), None,
                            op0=Alu.mult)
    c_sb = stp.tile([128, 3], f32, name="c_sb")
    nc.gpsimd.partition_broadcast(c_sb[:], vec3[0:1, :], channels=128)
    tau = c_sb[:, 0:1]
    neg_tau = c_sb[:, 1:2]
    ws127 = c_sb[:, 2:3]

    # ternary quantize the local shard (bit-exact; see module docstring)
    for i in range(NS):
        wa2 = wsb[i]  # shard raw strip is dead now; reuse as scratch
        nc.vector.tensor_scalar(wa2[:], wcen[i][:], neg_tau, -1.0,
                                op0=Alu.is_ge, op1=Alu.add)
        wqs = wpool.tile([128, D], bf16, name=f"wqs{i}", tag=f"wqs{i}")
        nc.vector.scalar_tensor_tensor(wqs[:], wcen[i][:], tau, wa2[:],
                                       op0=Alu.is_gt, op1=Alu.add)
        nc.scalar.dma_start(gql.ap()[i * 128:(i + 1) * 128, :], wqs[:])

    # gather the full ternary weight (bf16, [out_features, in_features])
    nc.gpsimd.collective_compute(
        "AllGather", Alu.bypass, replica_groups=[list(range(N_CORES))],
        ins=[gql.ap().opt()], outs=[gqa.ap().opt()])

    # transposed strip loads: wq[k] = [k-part 128, out 2048] bf16
    wq_pool = ctx.enter_context(tc.tile_pool(name="wqp", bufs=1))
    wq = []
    for k in range(NK):
        wqk = wq_pool.tile([128, D], bf16, name=f"wq{k}", tag=f"wq{k}")
        nc.scalar.dma_start_transpose(wqk[:],
                                    gqa.ap()[:, k * 128:(k + 1) * 128])
        wq.append(wqk)

    # ---- x pipeline pools ----
    x_pool = ctx.enter_context(tc.tile_pool(name="xp", bufs=3))
    st_pool = ctx.enter_context(tc.tile_pool(name="stp", bufs=6))
    xq_pool = ctx.enter_context(tc.tile_pool(name="xqp", bufs=3))
    xqT_pool = ctx.enter_context(tc.tile_pool(name="xqTp", bufs=4))
    tp_psum = ctx.enter_context(
        tc.tile_pool(name="tpps", bufs=2, space="PSUM"))
    y_psum = ctx.enter_context(
        tc.tile_pool(name="yps", bufs=6, space="PSUM"))
    y_pool = ctx.enter_context(tc.tile_pool(name="yop", bufs=2))

    def emit_x_prep(r):
        """Load + quantize + transpose one 128-row tile; returns (xqT, comb)."""
        xr = x_pool.tile([128, D], f32, name="xr")
        nc.sync.dma_start(xr[:], xs_ap[r * 128:(r + 1) * 128, :])

        mx = st_pool.tile([128, 1], f32, name="mx")
        nc.vector.tensor_reduce(mx[:], xr[:], axis=mybir.AxisListType.X,
                                op=Alu.max, apply_absolute_value=True)
        mxc = st_pool.tile([128, 1], f32, name="mxc")
        nc.vector.tensor_scalar(mxc[:], mx[:], 1e-5, None, op0=Alu.max)
        rec = st_pool.tile([128, 1], f32, name="rec")
        nc.vector.reciprocal(rec[:], mxc[:])
        comb = st_pool.tile([128, 1], f32, name="comb")
        nc.vector.tensor_scalar(comb[:], mxc[:], ws127, None, op0=Alu.mult)

        # xq = round_half_even(x * (1/s) * 127) in bf16
        xsc = x_pool.tile([128, D], f32, name="xsc")
        nc.vector.tensor_scalar(xsc[:], xr[:], rec, 127.0,
                                op0=Alu.mult, op1=Alu.mult)
        xq = xq_pool.tile([128, D], bf16, name="xq")
        nc.vector.tensor_scalar(xq[:], xsc[:], MAGIC, MAGIC,
                                op0=Alu.add, op1=Alu.subtract)

        # transpose xq via PE into xqT (bf16), strip by strip
        xqT = xqT_pool.tile([128, D], bf16, name="xqT")
        for h in range(2):
            pst = tp_psum.tile([128, 1024], bf16, name="pst")
            for kk in range(8):
                k = h * 8 + kk
                nc.tensor.transpose(pst[:, kk * 128:(kk + 1) * 128],
                                    xq[:, k * 128:(k + 1) * 128],
                                    ident[:])
            nc.scalar.copy(xqT[:, h * 1024:(h + 1) * 1024], pst[:])
        return xqT, comb

    def emit_mms(r, xqT, yps, ks):
        """k-outer / o-inner: one LDWEIGHTS per k feeds 4 o-bank matmuls."""
        for k in ks:
            for o in range(NO):
                nc.tensor.matmul(yps[o][:],
                                 xqT[:, k * 128:(k + 1) * 128],
                                 wq[k][:, o * 512:(o + 1) * 512],
                                 start=(k == 0), stop=(k == NK - 1))

    def emit_finish(r, yps, comb):
        ysb = y_pool.tile([128, D], f32, name="ysb")
        for o in range(NO):
            # dequant rescale: y * (w_scale * x_scale / 127)
            nc.scalar.mul(ysb[:, o * 512:(o + 1) * 512], yps[o][:], comb)
        nc.sync.dma_start(ys_ap[r * 128:(r + 1) * 128, :], ysb[:])

    def alloc_psum(r):
        # yp0/yp1 double-buffered so the next r-tile's first matmuls can
        # start while this one's rescales drain: 2+2+1+1 (+2 pst) = 8 banks
        return [y_psum.tile([128, 512], f32, name=f"yp{o}", tag=f"yp{o}",
                            bufs=(2 if o < 2 else 1))
                for o in range(NO)]

    # ---- prologue: r0/r1 x-prep first (overlaps the weight/collective ----
    # ---- path), then r0's and r1's matmuls as wq strips land          ----
    xqT0, comb0 = emit_x_prep(0)
    xqT1, comb1 = emit_x_prep(1)
    yps0 = alloc_psum(0)
    yps1 = alloc_psum(1)
    for k in range(NK):
        emit_mms(0, xqT0, yps0, [k])
        for o in range(2):  # r1 uses the double-buffered yp0/yp1 slots
            nc.tensor.matmul(yps1[o][:],
                             xqT1[:, k * 128:(k + 1) * 128],
                             wq[k][:, o * 512:(o + 1) * 512],
                             start=(k == 0), stop=(k == NK - 1))
    xqT2, comb2 = emit_x_prep(2)
    emit_finish(0, yps0, comb0)
    # r1's remaining o-banks (2nd k-pass), then finish r1
    for k in range(NK):
        for o in range(2, NO):
            nc.tensor.matmul(yps1[o][:],
                             xqT1[:, k * 128:(k + 1) * 128],
                             wq[k][:, o * 512:(o + 1) * 512],
                             start=(k == 0), stop=(k == NK - 1))
    xqT3, comb3 = emit_x_prep(3)
    emit_finish(1, yps1, comb1)

    # ---- steady state: two-stage software pipeline ----
    pending = [(2, xqT2, comb2), (3, xqT3, comb3)]
    for r in range(4, NR + 2):
        pr, pxqT, pcomb = pending.pop(0)
        yps = alloc_psum(pr)
        emit_mms(pr, pxqT, yps, range(NK))
        if r < NR:
            pending.append((r, *emit_x_prep(r)))
        emit_finish(pr, yps, pcomb)


def _build_program():
    import concourse.bacc as bacc
    import concourse.mybir as mybir
    import concourse.tile as tile
    from contextlib import ExitStack

    f32 = mybir.dt.float32
    bf16 = mybir.dt.bfloat16
    nc = bacc.Bacc("TRN2", target_bir_lowering=False, debug=False,
                   num_devices=N_CORES)

    xs = nc.dram_tensor("xs", [R_CORE, D], f32, kind="ExternalInput")
    wt = nc.dram_tensor("wt", [O_SHARD, D], f32, kind="ExternalInput")
    ys = nc.dram_tensor("ys", [R_CORE, D], f32, kind="ExternalOutput")
    gql = nc.dram_tensor("gql", [O_SHARD, D], bf16, kind="Internal")
    gqa = nc.dram_tensor("gqa", [D, D], bf16, kind="Internal",
                         addr_space="Shared")

    with tile.TileContext(nc) as tc, ExitStack() as ctx:
        _emit(nc, tc, ctx, xs.ap(), wt.ap(), ys.ap(), gql, gqa)

    nc.compile()
    return nc


def _build_dispatch():
    """jit(shard_map) wrapper around bass_exec: global arrays sharded on
    axis 0, no host-side concat, no donated zero output buffers."""
    import jax
    from jax.sharding import Mesh, PartitionSpec
    from jax.experimental.shard_map import shard_map
    import concourse.mybir as mybir
    from concourse import bass2jax

    global _NC
    nc = _build_program()
    _NC = nc

    partition_name = (nc.partition_id_tensor.name
                      if nc.partition_id_tensor else None)
    in_names, out_names, out_avals = [], [], []
    for alloc in nc.m.functions[0].allocations:
        if not isinstance(alloc, mybir.MemoryLocationSet):
            continue
        name = alloc.memorylocations[0].name
        if alloc.kind == "ExternalInput":
            if name != partition_name:
                in_names.append(name)
        elif alloc.kind == "ExternalOutput":
            out_names.append(name)
            out_avals.append(jax.core.ShapedArray(
                tuple(alloc.tensor_shape), mybir.dt.np(alloc.dtype)))
    if partition_name is not None:
        in_names.append(partition_name)
    assert in_names[:2] == ["xs", "wt"] and out_names == ["ys"], (
        in_names, out_names)

    bass2jax.install_neuronx_cc_hook()

    def _body(*args):
        operands = list(args)
        if partition_name is not None:
            operands.append(bass2jax.partition_id_tensor())
        outs = bass2jax._bass_exec_p.bind(
            *operands,
            out_avals=tuple(out_avals),
            in_names=tuple(in_names),
            out_names=tuple(out_names),
            lowering_input_output_aliases=(),
            sim_require_finite=True,
            sim_require_nnan=True,
            nc=nc,
        )
        return tuple(outs)

    devices = jax.devices()[:N_CORES]
    mesh = Mesh(np.asarray(devices), ("core",))
    spec = PartitionSpec("core")

    def _make_jit():
        return jax.jit(shard_map(_body, mesh=mesh, in_specs=(spec, spec),
                                 out_specs=(spec,), check_rep=False))

    try:
        # AOT-compile with bass_effect suppressed: the jit call takes the
        # C++ fast dispatch path instead of the python effects path.
        from jax.sharding import NamedSharding
        sh = NamedSharding(mesh, spec)
        x_sds = jax.ShapeDtypeStruct((R_TOTAL, D), np.float32, sharding=sh)
        w_sds = jax.ShapeDtypeStruct((D, D), np.float32, sharding=sh)
        return bass2jax.fast_dispatch_compile(
            lambda: _make_jit().lower(x_sds, w_sds).compile())
    except Exception:
        return _make_jit()


def _get_fn():
    key = (R_CORE, D)
    if key not in _FN_CACHE:
        _FN_CACHE[key] = _build_dispatch()
    return _FN_CACHE[key]


def kernel(x: np.ndarray, weight: np.ndarray, _trace: bool = False,
           **_unused) -> np.ndarray:
    global LAST_RESULTS
    LAST_RESULTS = None

    x = np.asarray(x)
    weight = np.asarray(weight)
    orig_shape = x.shape
    x2d = np.ascontiguousarray(
        x.reshape(R_TOTAL, D).astype(np.float32, copy=False))
    w = np.ascontiguousarray(weight.astype(np.float32, copy=False))

    fn = _get_fn()
    out = fn(x2d, w)[0]
    y2d = np.asarray(out)
    return y2d.reshape(orig_shape).astype(np.float32, copy=False)
